# revision 24
# baseline (speedup 1.0000x reference)
"""GATv2 4-layer GNN on 8 TRN2 NeuronCores.

Sharding: nodes LPT-packed into 240 balanced (core,window) bins of 128 nodes;
edges assigned to the bin of their dst. Per layer: local node matmuls ->
AllGather of bf16 xl -> per-window edge phase (indirect-DMA gather of xl[src],
one-hot matmuls for xr expansion and segment-softmax scatter into PSUM) ->
BN-stat AllReduce -> fused BN+elu. Final: graph mean-pool via indicator
matmuls + AllReduce + 2-layer MLP (replicated).
"""
import heapq

import numpy as np
import ml_dtypes

import concourse.bass as bass
import concourse.bacc as bacc
import concourse.mybir as mybir
import concourse.tile as tile
from concourse.bass_utils import run_bass_kernel_spmd
from concourse.masks import make_identity

P = 128
NCORE = 8
N = 30000
E = 300000
G = 128
IN = 128
HID = 64
HEADS = 4
NCLS = 10
EPS = 1e-5
SLOPE = 0.2
NPC = N // NCORE            # 3750 real nodes per core
W = 30                      # windows per core
NPAD = W * P                # 3840 padded nodes per core
ROWS = NCORE * NPAD         # 30720 rows in the allgathered xl
OC = HEADS * HID            # 256
# (layer, input-channel-chunk) order for weight / hT chunk layouts
IC_CHUNKS = [(0, 0), (1, 0), (1, 1), (2, 0), (2, 1), (3, 0), (3, 1)]
# (layer, output-channel-chunk) order for BN params/stats
OC_CHUNKS = [(0, 0), (0, 1), (1, 0), (1, 1), (2, 0), (2, 1), (3, 0)]

f32 = mybir.dt.float32
bf16 = mybir.dt.bfloat16
i32 = mybir.dt.int32
bf = ml_dtypes.bfloat16


def _pack_nodes(deg):
    """LPT-pack nodes into NCORE*W bins of <=128 nodes, balancing edge load."""
    nbins = NCORE * W
    order = np.argsort(-deg, kind="stable")
    heap = [(0, b) for b in range(nbins)]
    heapq.heapify(heap)
    counts = np.zeros(nbins, np.int64)
    load = np.zeros(nbins, np.int64)
    node_bin = np.empty(N, np.int64)
    node_pos = np.empty(N, np.int64)
    for n in order:
        while True:
            l, b = heapq.heappop(heap)
            if counts[b] < P:
                break
        node_bin[n] = b
        node_pos[n] = counts[b]
        counts[b] += 1
        load[b] += deg[n]
        if counts[b] < P:
            heapq.heappush(heap, (load[b], b))
    return node_bin, node_pos, int(load.max())


def _host_prep(inputs):
    x = np.asarray(inputs["x"], np.float32)
    ei = np.asarray(inputs["edge_index"], np.int64)
    batch = np.asarray(inputs["batch"], np.int64)

    src = ei[0]
    dst = ei[1]
    deg = np.bincount(dst, minlength=N) + 1  # +1 self-loop
    node_bin, node_pos, max_load = _pack_nodes(deg)
    nodes_per_bin = np.bincount(node_bin, minlength=NCORE * W)
    nonself_load = np.bincount(node_bin[dst], minlength=NCORE * W)
    cpw = 1 + max(1, -(-int(nonself_load.max()) // P))  # chunk0 = self loops
    nch = W * cpw                            # chunk columns per core
    row_of = (node_bin // W) * NPAD + (node_bin % W) * P + node_pos  # global row

    # per-edge (non-self): assigned bin = bin of dst
    e_bin = node_bin[dst]
    e_srcrow = row_of[src].astype(np.int32)
    e_dstpos = node_pos[dst].astype(np.int32)
    order = np.argsort(e_bin, kind="stable")
    sorted_bin = e_bin[order]
    counts_b = np.bincount(sorted_bin, minlength=NCORE * W)

    srcrow_tab = np.zeros((NCORE, P, nch), np.int32)
    dstf_tab = np.full((NCORE, P, nch), 999.0, bf)

    # chunk 0 of each window: self loop of node at position p (pad -> masked)
    cores_n = node_bin // W
    wins_n = node_bin % W
    srcrow_tab[cores_n, node_pos, wins_n * cpw] = row_of[np.arange(N)]
    dstf_tab[cores_n, node_pos, wins_n * cpw] = node_pos
    selfb_tab = np.full((NCORE, P, W), -1000.0, np.float32)
    selfb_tab[cores_n, node_pos, wins_n] = 0.0

    bin_starts = np.zeros(NCORE * W + 1, np.int64)
    np.cumsum(counts_b, out=bin_starts[1:])
    se_srcrow = e_srcrow[order]
    se_dstpos = e_dstpos[order]
    for b in range(NCORE * W):
        c, w = b // W, b % W
        lo, hi = bin_starts[b], bin_starts[b + 1]
        k = hi - lo
        sl = np.arange(k)
        ch = w * cpw + 1 + sl // P    # chunk column (after self chunk)
        pp = sl % P                   # partition
        srcrow_tab[c, pp, ch] = se_srcrow[lo:hi]
        dstf_tab[c, pp, ch] = se_dstpos[lo:hi]

    # graph indicator per core: [P pos, W*P] (win w block cols -> graph onehot)
    ind_tab = np.zeros((NCORE, P, W * P), bf)
    ind_tab[cores_n, node_pos, wins_n * P + batch] = 1.0

    # x transposed per core, permuted order: [IN, NPAD]
    xT = np.zeros((NCORE, IN, NPAD), bf)
    cols = wins_n * P + node_pos
    xT[cores_n, :, cols] = x.astype(bf)

    # weights (identical on all cores)
    wl_tab = np.zeros((P, len(IC_CHUNKS) * OC), bf)
    wr_tab = np.zeros((P, len(IC_CHUNKS) * OC), bf)
    for j, (l, cc) in enumerate(IC_CHUNKS):
        wl = np.asarray(inputs[f"Wl{l}"], np.float32)
        wr = np.asarray(inputs[f"Wr{l}"], np.float32)
        wl_tab[:, j * OC:(j + 1) * OC] = wl[cc * P:(cc + 1) * P, :].astype(bf)
        wr_tab[:, j * OC:(j + 1) * OC] = wr[cc * P:(cc + 1) * P, :].astype(bf)
    att_tab = np.zeros((P, 4 * 2 * OC), bf)
    for l in range(4):
        a = np.asarray(inputs[f"att{l}"], np.float32).reshape(1, OC)
        att_tab[:, l * 2 * OC:(l + 1) * 2 * OC] = np.tile(a, (P, 2))
    g_tab = np.ones((P, len(OC_CHUNKS)), np.float32)
    b_tab = np.zeros((P, len(OC_CHUNKS)), np.float32)
    for j, (l, cc) in enumerate(OC_CHUNKS):
        g = np.asarray(inputs[f"bng{l}"], np.float32)
        bb = np.asarray(inputs[f"bnb{l}"], np.float32)
        seg = g[cc * P:(cc + 1) * P]
        g_tab[: len(seg), j] = seg
        segb = bb[cc * P:(cc + 1) * P]
        b_tab[: len(segb), j] = segb

    fc1w = np.zeros((HID, HID), bf)
    fc1w[:, :] = np.asarray(inputs["fc1_w"], np.float32).astype(bf)
    fc2w = np.zeros((HID, NCLS), bf)
    fc2w[:, :] = np.asarray(inputs["fc2_w"], np.float32).astype(bf)
    fc1b_rep = np.tile(np.asarray(inputs["fc1_b"], np.float32).reshape(1, HID), (P, 1))
    fc2b_rep = np.tile(np.asarray(inputs["fc2_b"], np.float32).reshape(1, NCLS), (P, 1))
    gcnt = np.bincount(batch, minlength=G).astype(np.float32)
    cnt_recip = (1.0 / np.maximum(gcnt, 1.0)).reshape(P, 1).astype(np.float32)

    in_maps = []
    for c in range(NCORE):
        in_maps.append(dict(
            xT=np.ascontiguousarray(xT[c]),
            selfb_tab=np.ascontiguousarray(selfb_tab[c]),
            srcrow_tab=np.ascontiguousarray(srcrow_tab[c]),
            dstf_tab=np.ascontiguousarray(dstf_tab[c]),
            ind_tab=np.ascontiguousarray(ind_tab[c]),
            wl_tab=wl_tab, wr_tab=wr_tab, att_tab=att_tab,
            g_tab=g_tab, b_tab=b_tab,
            fc1w=fc1w, fc2w=fc2w, fc1b_rep=fc1b_rep, fc2b_rep=fc2b_rep,
            cnt_recip=cnt_recip,
        ))
    return in_maps, cpw


def _build_program(cpw):
    nch = W * cpw
    nc = bacc.Bacc(num_devices=NCORE, num_swdge_queues=4,
                   dynamic_dma_scratch_size=32768)
    ap = mybir.AluOpType
    act = mybir.ActivationFunctionType
    rg = [list(range(NCORE))]

    # --- external inputs ---
    xT_in = nc.dram_tensor("xT", [IN, NPAD], bf16, kind="ExternalInput")
    srcrow_in = nc.dram_tensor("srcrow_tab", [P, nch], i32, kind="ExternalInput")
    selfb_in = nc.dram_tensor("selfb_tab", [P, W], f32, kind="ExternalInput")
    dstf_in = nc.dram_tensor("dstf_tab", [P, nch], bf16, kind="ExternalInput")
    ind_in = nc.dram_tensor("ind_tab", [P, W * P], bf16, kind="ExternalInput")
    wl_in = nc.dram_tensor("wl_tab", [P, len(IC_CHUNKS) * OC], bf16, kind="ExternalInput")
    wr_in = nc.dram_tensor("wr_tab", [P, len(IC_CHUNKS) * OC], bf16, kind="ExternalInput")
    att_in = nc.dram_tensor("att_tab", [P, 8 * OC], bf16, kind="ExternalInput")
    g_in = nc.dram_tensor("g_tab", [P, len(OC_CHUNKS)], f32, kind="ExternalInput")
    b_in = nc.dram_tensor("b_tab", [P, len(OC_CHUNKS)], f32, kind="ExternalInput")
    fc1w_in = nc.dram_tensor("fc1w", [HID, HID], bf16, kind="ExternalInput")
    fc2w_in = nc.dram_tensor("fc2w", [HID, NCLS], bf16, kind="ExternalInput")
    fc1b_in = nc.dram_tensor("fc1b_rep", [P, HID], f32, kind="ExternalInput")
    fc2b_in = nc.dram_tensor("fc2b_rep", [P, NCLS], f32, kind="ExternalInput")
    cnt_in = nc.dram_tensor("cnt_recip", [P, 1], f32, kind="ExternalInput")
    out_dram = nc.dram_tensor("out", [G, NCLS], f32, kind="ExternalOutput")

    with tile.TileContext(nc) as tc:
        with (
            tc.tile_pool(name="persist", bufs=1) as pers,
            tc.tile_pool(name="sbuf", bufs=3) as sb,
            tc.tile_pool(name="psum", bufs=2, space="PSUM") as ps,
            tc.tile_pool(name="psacc", bufs=3, space="PSUM") as psacc,
            tc.tile_pool(name="dram", bufs=1, space="DRAM") as dr,
        ):
            # ---- persistent tiles / table loads ----
            ident_f = pers.tile([P, P], f32)
            make_identity(nc, ident_f[:])
            ident_bf = pers.tile([P, P], bf16)
            nc.vector.tensor_copy(ident_bf[:], ident_f[:])
            iota_row_i = pers.tile([P, P], i32)
            nc.gpsimd.iota(iota_row_i[:], pattern=[[1, P]], base=0, channel_multiplier=0)
            iota_row_f = pers.tile([P, P], bf16)
            nc.vector.tensor_copy(iota_row_f[:], iota_row_i[:])
            iota_col_i = pers.tile([P, 1], i32)
            nc.gpsimd.iota(iota_col_i[:], pattern=[[0, 1]], base=0, channel_multiplier=1)
            iota_col_f = pers.tile([P, 1], f32)
            nc.vector.tensor_copy(iota_col_f[:], iota_col_i[:])
            ones_row = pers.tile([1, P], bf16)
            nc.vector.memset(ones_row[:], 1.0)
            alpha_col = pers.tile([P, 1], f32)
            nc.vector.memset(alpha_col[:], SLOPE)
            ones_f = pers.tile([1, P], f32)
            nc.vector.memset(ones_f[:], 1.0)

            def load(name, handle, shape, dt):
                t = pers.tile(shape, dt, name=name)
                nc.sync.dma_start(t[:], handle[:, :])
                return t
            xT_sb = load("xT_sb", xT_in, [IN, NPAD], bf16)
            srcrow = load("srcrow", srcrow_in, [P, nch], i32)
            selfb = load("selfb", selfb_in, [P, W], f32)
            dstf = load("dstf", dstf_in, [P, nch], bf16)
            ind_sb = load("ind_sb", ind_in, [P, W * P], bf16)
            wl_sb = load("wl_sb", wl_in, [P, len(IC_CHUNKS) * OC], bf16)
            wr_sb = load("wr_sb", wr_in, [P, len(IC_CHUNKS) * OC], bf16)
            att_sb = load("att_sb", att_in, [P, 8 * OC], bf16)
            g_sb = load("g_sb", g_in, [P, len(OC_CHUNKS)], f32)
            b_sb = load("b_sb", b_in, [P, len(OC_CHUNKS)], f32)
            fc1w_sb = load("fc1w_sb", fc1w_in, [HID, HID], bf16)
            fc2w_sb = load("fc2w_sb", fc2w_in, [HID, NCLS], bf16)
            fc1b_sb = load("fc1b_sb", fc1b_in, [P, HID], f32)
            fc2b_sb = load("fc2b_sb", fc2b_in, [P, NCLS], f32)
            cnt_sb = load("cnt_sb", cnt_in, [P, 1], f32)

            hT_a = pers.tile([P, NPAD], bf16)      # h^T chunk c0:128
            hT_b = pers.tile([P, NPAD], bf16)      # h^T chunk c128:256
            hpre_a = pers.tile([P, NPAD], bf16)    # pre-BN h^T chunk 0
            hpre_b = pers.tile([P, NPAD], bf16)
            h3_pre = pers.tile([P, W * HID], f32)  # layer3 pre-BN, [n, c] layout
            h3_bn = pers.tile([P, W * HID], bf16)
            xr_all = pers.tile([P, W * OC], bf16)
            stat_s = pers.tile([P, 2 * W], f32)    # per-window sums, cc in {0,1}
            stat_q = pers.tile([P, 2 * W], f32)    # per-window sum-of-squares

            # DRAM scratch
            xl_loc = dr.tile([NPAD, OC], bf16)
            xl_fulls = [dr.tile([ROWS, OC], bf16, addr_space="Shared", name=f"xl_full{i}")
                        for i in range(4)]
            stats_i = dr.tile([P, 4], f32)
            stats_os = [dr.tile([P, 4], f32, addr_space="Shared", name=f"stats_o{i}")
                        for i in range(4)]
            pool_i = dr.tile([P, HID], f32)
            pool_o = dr.tile([P, HID], f32, addr_space="Shared")

            for l in range(4):
                ic_chunks = [j for j, (ll, _) in enumerate(IC_CHUNKS) if ll == l]
                oc_chunks = [j for j, (ll, _) in enumerate(OC_CHUNKS) if ll == l]
                hts = [xT_sb] if l == 0 else [hT_a, hT_b]

                # ---- node phase: xl = h @ Wl, xr = h @ Wr ----
                for w in range(W):
                    ps_xl = ps.tile([P, OC], f32, name="ps_xl", tag="psA")
                    ps_xr = ps.tile([P, OC], f32, name="ps_xr", tag="psB")
                    for i, j in enumerate(ic_chunks):
                        lhs = hts[i][:, w * P:(w + 1) * P]
                        nc.tensor.matmul(ps_xl[:], lhsT=lhs, rhs=wl_sb[:, j * OC:(j + 1) * OC],
                                         start=(i == 0), stop=(i == len(ic_chunks) - 1))
                        nc.tensor.matmul(ps_xr[:], lhsT=lhs, rhs=wr_sb[:, j * OC:(j + 1) * OC],
                                         start=(i == 0), stop=(i == len(ic_chunks) - 1))
                    xl_w = sb.tile([P, OC], bf16, name="xl_w")
                    nc.scalar.copy(xl_w[:], ps_xl[:])
                    nc.sync.dma_start(xl_loc[w * P:(w + 1) * P, :], xl_w[:])
                    nc.scalar.copy(xr_all[:, w * OC:(w + 1) * OC], ps_xr[:])

                # ---- allgather xl ----
                xl_full = xl_fulls[l]
                nc.gpsimd.collective_compute(
                    "AllGather", ap.bypass, replica_groups=rg,
                    ins=[xl_loc[:]], outs=[xl_full[:]],
                )

                # ---- edge phase ----
                attl = att_sb[:, l * 2 * OC:l * 2 * OC + OC]
                att2l = att_sb[:, l * 2 * OC:l * 2 * OC + 2 * OC]
                ngr = (cpw - 1) // 2
                for w in range(W):
                    acc = psacc.tile([P, OC + 4], f32, name="acc", tag="acc")
                    xrw = xr_all[:, w * OC:(w + 1) * OC]
                    # ---- chunk 0: self loops (identity one-hots) ----
                    kk0 = w * cpw
                    A0 = sb.tile([P, OC], bf16, name="A0", bufs=3)
                    nc.sync.dma_start(A0[:], xl_loc[w * P:(w + 1) * P, :])
                    z0 = sb.tile([P, OC], f32, name="z0", bufs=3)
                    nc.vector.tensor_tensor(out=z0[:], in0=A0[:], in1=xrw, op=ap.add)
                    z0l = sb.tile([P, OC], bf16, name="z0l", bufs=3)
                    nc.scalar.activation(z0l[:], z0[:], act.Prelu,
                                         bias=0.0, scale=1.0, alpha=alpha_col[:, :1])
                    zw0 = sb.tile([P, OC], bf16, name="zw0", bufs=3)
                    nc.vector.tensor_tensor(out=zw0[:], in0=z0l[:], in1=attl, op=ap.mult)
                    lg0 = sb.tile([P, 4], f32, name="lg0", bufs=3)
                    nc.vector.tensor_reduce(
                        out=lg0[:], in_=zw0[:].rearrange("p (h c) -> p h c", h=4),
                        axis=mybir.AxisListType.X, op=ap.add)
                    pa0 = sb.tile([P, OC + 4], bf16, name="pa0", bufs=3)
                    nc.scalar.activation(pa0[:, OC:OC + 4], lg0[:], act.Exp,
                                         bias=selfb[:, w:w + 1], scale=1.0)
                    nc.vector.tensor_tensor(
                        out=pa0[:, 0:OC].rearrange("p (h c) -> p h c", h=4),
                        in0=A0[:].rearrange("p (h c) -> p h c", h=4),
                        in1=pa0[:, OC:OC + 4][:, :, None].to_broadcast([P, 4, HID]),
                        op=ap.mult)
                    nc.tensor.matmul(acc[:], lhsT=ident_bf[:], rhs=pa0[:],
                                     start=True, stop=False)
                    # ---- groups of 2 regular chunks ----
                    for g in range(ngr):
                        kka = w * cpw + 1 + 2 * g
                        A2 = sb.tile([P, 2 * OC], bf16, name="A2", bufs=16)
                        psz = ps.tile([P, 2 * OC], f32, name="psz", tag="psA")
                        scs = []
                        for j in range(2):
                            kk = kka + j
                            g_inst = nc.gpsimd.indirect_dma_start(
                                out=A2[:, j * OC:(j + 1) * OC], out_offset=None,
                                in_=xl_full[:, :],
                                in_offset=bass.IndirectOffsetOnAxis(
                                    ap=srcrow[:, kk:kk + 1], axis=0),
                            )
                            qn = kk % 4
                            g_inst.ins.queue = f"qPoolDynamic{qn if qn else ''}"
                            sc_oh = sb.tile([P, P], bf16, name=f"sc_oh{j}", bufs=6)
                            nc.vector.tensor_tensor(
                                out=sc_oh[:], in0=dstf[:, kk:kk + 1].to_broadcast([P, P]),
                                in1=iota_row_f[:], op=ap.is_equal)
                            ps_dT = ps.tile([P, P], bf16, name="ps_dT", tag="psB")
                            nc.tensor.transpose(ps_dT[:], sc_oh[:], ident_bf[:])
                            ex_oh = sb.tile([P, P], bf16, name=f"ex_oh{j}", bufs=6)
                            nc.scalar.copy(ex_oh[:], ps_dT[:])
                            zsl = slice(j * OC, (j + 1) * OC)
                            nc.tensor.matmul(psz[:, zsl], lhsT=ex_oh[:], rhs=xrw,
                                             start=True, stop=False)
                            nc.tensor.matmul(psz[:, zsl], lhsT=ident_bf[:],
                                             rhs=A2[:, zsl], start=False, stop=True)
                            scs.append(sc_oh)
                        zl2 = sb.tile([P, 2 * OC], bf16, name="zl2", bufs=4)
                        nc.scalar.activation(zl2[:], psz[:], act.Prelu,
                                             bias=0.0, scale=1.0, alpha=alpha_col[:, :1])
                        zw2 = sb.tile([P, 2 * OC], bf16, name="zw2", bufs=3)
                        nc.vector.tensor_tensor(out=zw2[:], in0=zl2[:], in1=att2l, op=ap.mult)
                        lg2 = sb.tile([P, 8], f32, name="lg2", bufs=3)
                        nc.vector.tensor_reduce(
                            out=lg2[:], in_=zw2[:].rearrange("p (g h c) -> p g h c", g=2, h=4),
                            axis=mybir.AxisListType.X, op=ap.add)
                        pa2 = sb.tile([P, 2 * (OC + 4)], bf16, name="pa2", bufs=6)
                        pa2v = pa2[:].rearrange("p (g d) -> p g d", g=2)
                        nc.scalar.activation(
                            pa2v[:, :, OC:OC + 4], lg2[:].rearrange("p (g h) -> p g h", g=2),
                            act.Exp)
                        for j in range(2):
                            nc.vector.tensor_tensor(
                                out=pa2v[:, j, 0:OC].rearrange("p (h c) -> p h c", h=4),
                                in0=A2[:, j * OC:(j + 1) * OC]
                                    .rearrange("p (h c) -> p h c", h=4),
                                in1=pa2v[:, j, OC:OC + 4][:, :, None]
                                    .to_broadcast([P, 4, HID]),
                                op=ap.mult)
                            nc.tensor.matmul(
                                acc[:], lhsT=scs[j],
                                rhs=pa2v[:, j, :],
                                start=False,
                                stop=(g == ngr - 1 and j == 1))
                    # ---- window finalize ----
                    s_cl = sb.tile([P, 4], f32, name="s_cl")
                    nc.vector.tensor_scalar_max(s_cl[:], acc[:, OC:OC + 4], 1e-30)
                    r_t = sb.tile([P, 4], f32, name="r_t")
                    nc.vector.reciprocal(r_t[:], s_cl[:])
                    o_sb = sb.tile([P, OC], f32, name="o_sb")
                    nc.vector.tensor_tensor(
                        out=o_sb[:].rearrange("p (h c) -> p h c", h=4),
                        in0=acc[:, 0:OC].rearrange("p (h c) -> p h c", h=4),
                        in1=r_t[:, :, None].to_broadcast([P, 4, HID]),
                        op=ap.mult)
                    if l < 3:
                        for cc, hp in enumerate([hpre_a, hpre_b]):
                            ps_t = ps.tile([P, P], f32, name="ps_t", tag="psC", bufs=1)
                            nc.tensor.transpose(ps_t[:], o_sb[:, cc * P:(cc + 1) * P], ident_f[:])
                            nc.scalar.activation(
                                hp[:, w * P:(w + 1) * P], ps_t[:], act.Identity,
                                accum_out=stat_s[:, cc * W + w:cc * W + w + 1])
                            sq_t = sb.tile([P, P], f32, name="sq_t")
                            nc.scalar.activation(
                                sq_t[:], ps_t[:], act.Square,
                                accum_out=stat_q[:, cc * W + w:cc * W + w + 1])
                    else:
                        u1 = sb.tile([P, HID], f32, name="u1")
                        nc.vector.tensor_tensor(out=u1[:], in0=o_sb[:, 0:HID],
                                                in1=o_sb[:, HID:2 * HID], op=ap.add)
                        u2 = sb.tile([P, HID], f32, name="u2")
                        nc.vector.tensor_tensor(out=u2[:], in0=o_sb[:, 2 * HID:3 * HID],
                                                in1=o_sb[:, 3 * HID:4 * HID], op=ap.add)
                        u3 = sb.tile([P, HID], f32, name="u3")
                        nc.vector.tensor_tensor(out=u3[:], in0=u1[:], in1=u2[:], op=ap.add)
                        nc.vector.tensor_scalar_mul(
                            h3_pre[:, w * HID:(w + 1) * HID], u3[:], 0.25)
                        ps_t3 = ps.tile([HID, P], f32, name="ps_t3", tag="psC", bufs=1)
                        nc.tensor.transpose(ps_t3[:], h3_pre[:, w * HID:(w + 1) * HID], ident_f[:])
                        sq_t3 = sb.tile([HID, P], f32, name="sq_t3")
                        nc.scalar.activation(
                            sq_t3[:], ps_t3[:], act.Square,
                            accum_out=stat_q[:HID, w:w + 1])
                        nc.vector.tensor_reduce(
                            out=stat_s[:HID, w:w + 1], in_=ps_t3[:],
                            axis=mybir.AxisListType.X, op=ap.add)

                # ---- BN stats allreduce ----
                ncc = len(oc_chunks)
                st_sb = sb.tile([P, 4], f32, name="st_sb")
                if l < 3:
                    for cc in range(2):
                        nc.vector.tensor_reduce(
                            out=st_sb[:, cc:cc + 1], in_=stat_s[:, cc * W:(cc + 1) * W],
                            axis=mybir.AxisListType.X, op=ap.add)
                        nc.vector.tensor_reduce(
                            out=st_sb[:, 2 + cc:3 + cc], in_=stat_q[:, cc * W:(cc + 1) * W],
                            axis=mybir.AxisListType.X, op=ap.add)
                else:
                    nc.vector.tensor_reduce(
                        out=st_sb[:, 0:1], in_=stat_s[:, 0:W],
                        axis=mybir.AxisListType.X, op=ap.add)
                    nc.vector.tensor_reduce(
                        out=st_sb[:, 2:3], in_=stat_q[:, 0:W],
                        axis=mybir.AxisListType.X, op=ap.add)
                    nc.vector.memset(st_sb[:, 1:2], 0.0)
                    nc.vector.memset(st_sb[:, 3:4], 0.0)
                nc.sync.dma_start(stats_i[:], st_sb[:])
                nc.gpsimd.collective_compute(
                    "AllReduce", ap.add, replica_groups=rg,
                    ins=[stats_i[:]], outs=[stats_os[l][:]])
                gstats = sb.tile([P, 4], f32, name="gstats")
                nc.sync.dma_start(gstats[:], stats_os[l][:])

                # scale/shift: [P, 2] (cc cols)
                mu = sb.tile([P, 2], f32, name="mu")
                nc.vector.tensor_scalar_mul(mu[:], gstats[:, 0:2], 1.0 / N)
                msq = sb.tile([P, 2], f32, name="msq")
                nc.vector.tensor_scalar_mul(msq[:], gstats[:, 2:4], 1.0 / N)
                mu2 = sb.tile([P, 2], f32, name="mu2")
                nc.vector.tensor_tensor(out=mu2[:], in0=mu[:], in1=mu[:], op=ap.mult)
                var = sb.tile([P, 2], f32, name="var")
                nc.vector.tensor_tensor(out=var[:], in0=msq[:], in1=mu2[:], op=ap.subtract)
                vpe = sb.tile([P, 2], f32, name="vpe")
                nc.vector.tensor_scalar_add(vpe[:], var[:], EPS)
                rec = sb.tile([P, 2], f32, name="rec")
                nc.vector.reciprocal(rec[:], vpe[:])
                rstd = sb.tile([P, 2], f32, name="rstd")
                nc.scalar.sqrt(rstd[:], rec[:])
                scal = sb.tile([P, 2], f32, name="scal")
                shif = sb.tile([P, 2], f32, name="shif")
                for i, j in enumerate(oc_chunks):
                    nc.vector.tensor_tensor(out=scal[:, i:i + 1], in0=g_sb[:, j:j + 1],
                                            in1=rstd[:, i:i + 1], op=ap.mult)
                    tmp_ms = sb.tile([P, 1], f32, name="tmp_ms")
                    nc.vector.tensor_tensor(out=tmp_ms[:], in0=mu[:, i:i + 1],
                                            in1=scal[:, i:i + 1], op=ap.mult)
                    nc.vector.tensor_tensor(out=shif[:, i:i + 1], in0=b_sb[:, j:j + 1],
                                            in1=tmp_ms[:], op=ap.subtract)

                # ---- BN apply + elu ----
                if l < 3:
                    for cc, (hp, ht) in enumerate([(hpre_a, hT_a), (hpre_b, hT_b)]):
                        for hh in range(4):
                            hsl = slice(hh * (NPAD // 4), (hh + 1) * (NPAD // 4))
                            t_big = sb.tile([P, NPAD // 4], f32, name="t_big", bufs=2)
                            nc.vector.tensor_scalar(
                                out=t_big[:], in0=hp[:, hsl], scalar1=scal[:, cc:cc + 1],
                                scalar2=shif[:, cc:cc + 1], op0=ap.mult, op1=ap.add)
                            m_big = sb.tile([P, NPAD // 4], f32, name="m_big", bufs=2)
                            nc.vector.tensor_scalar_min(m_big[:], t_big[:], 0.0)
                            nc.scalar.activation(m_big[:], m_big[:], act.Exp)
                            nc.vector.tensor_scalar_add(m_big[:], m_big[:], -1.0)
                            nc.vector.tensor_tensor(out=ht[:, hsl], in0=t_big[:], in1=m_big[:], op=ap.max)
                else:
                    # replicate scale/shift rows: [P,1]->[1,P]->K=1 matmul
                    for nm, col in (("scal3", scal), ("shif3", shif)):
                        ps_r = ps.tile([1, P], f32, name="ps_r", tag="psC", bufs=1)
                        nc.tensor.transpose(ps_r[:], col[:, 0:1], ident_f[:])
                        row_t = sb.tile([1, P], f32, name=nm + "_row")
                        nc.scalar.copy(row_t[:], ps_r[:])
                        ps_rep = ps.tile([P, HID], f32, name="ps_rep", tag="psB")
                        nc.tensor.matmul(ps_rep[:], lhsT=ones_f[:, :P],
                                         rhs=row_t[:, 0:HID], start=True, stop=True)
                        rep_t = sb.tile([P, HID], f32, name=nm + "_rep", bufs=1)
                        nc.scalar.copy(rep_t[:], ps_rep[:])
                        if nm == "scal3":
                            scal3_rep = rep_t
                        else:
                            shif3_rep = rep_t
                    for w in range(W):
                        sl3 = slice(w * HID, (w + 1) * HID)
                        t3 = sb.tile([P, HID], f32, name="t3")
                        nc.vector.tensor_tensor(out=t3[:], in0=h3_pre[:, sl3],
                                                in1=scal3_rep[:], op=ap.mult)
                        nc.vector.tensor_tensor(out=t3[:], in0=t3[:],
                                                in1=shif3_rep[:], op=ap.add)
                        m3 = sb.tile([P, HID], f32, name="m3")
                        nc.vector.tensor_scalar_min(m3[:], t3[:], 0.0)
                        e3 = sb.tile([P, HID], f32, name="e3")
                        nc.scalar.activation(e3[:], m3[:], act.Exp)
                        nc.vector.tensor_scalar_add(e3[:], e3[:], -1.0)
                        nc.vector.tensor_tensor(out=h3_bn[:, sl3], in0=t3[:],
                                                in1=e3[:], op=ap.max)

            # ---- graph mean pool + MLP (replicated) ----
            ps_pool = psacc.tile([P, HID], f32, name="ps_pool", tag="acc")
            for w in range(W):
                nc.tensor.matmul(ps_pool[:], lhsT=ind_sb[:, w * P:(w + 1) * P],
                                 rhs=h3_bn[:, w * HID:(w + 1) * HID],
                                 start=(w == 0), stop=(w == W - 1))
            pool_sb = sb.tile([P, HID], f32, name="pool_sb")
            nc.vector.tensor_copy(pool_sb[:], ps_pool[:])
            nc.sync.dma_start(pool_i[:], pool_sb[:])
            nc.gpsimd.collective_compute(
                "AllReduce", ap.add, replica_groups=rg,
                ins=[pool_i[:]], outs=[pool_o[:]])
            pool_g = sb.tile([P, HID], f32, name="pool_g")
            nc.sync.dma_start(pool_g[:], pool_o[:])
            pooled = sb.tile([P, HID], f32, name="pooled")
            nc.vector.tensor_scalar_mul(pooled[:], pool_g[:], cnt_sb[:, :1])
            ps_pT = ps.tile([HID, P], f32, name="ps_pT", tag="psC", bufs=1)
            nc.tensor.transpose(ps_pT[:], pooled[:], ident_f[:])
            pooledT = sb.tile([HID, P], bf16, name="pooledT")
            nc.scalar.copy(pooledT[:], ps_pT[:])
            ps_o1 = ps.tile([P, HID], f32, name="ps_o1", tag="psA")
            nc.tensor.matmul(ps_o1[:], lhsT=pooledT[:], rhs=fc1w_sb[:, :], start=True, stop=True)
            o1b = sb.tile([P, HID], f32, name="o1b")
            nc.vector.tensor_tensor(out=o1b[:], in0=ps_o1[:], in1=fc1b_sb[:], op=ap.add)
            o1r = sb.tile([P, HID], bf16, name="o1r")
            nc.scalar.activation(o1r[:], o1b[:], act.Relu)
            o1rf = sb.tile([P, HID], f32, name="o1rf")
            nc.vector.tensor_copy(o1rf[:], o1r[:])
            ps_o1T = ps.tile([HID, P], f32, name="ps_o1T", tag="psB")
            nc.tensor.transpose(ps_o1T[:], o1rf[:], ident_f[:])
            o1T = sb.tile([HID, P], bf16, name="o1T")
            nc.scalar.copy(o1T[:], ps_o1T[:])
            ps_o2 = ps.tile([P, NCLS], f32, name="ps_o2", tag="psC", bufs=1)
            nc.tensor.matmul(ps_o2[:], lhsT=o1T[:], rhs=fc2w_sb[:, :], start=True, stop=True)
            o2b = sb.tile([P, NCLS], f32, name="o2b")
            nc.vector.tensor_tensor(out=o2b[:], in0=ps_o2[:], in1=fc2b_sb[:], op=ap.add)
            nc.sync.dma_start(out_dram[:, :], o2b[:])

    nc.compile()
    return nc


_PROG_CACHE = {}


def kernel(_trace=False, _tracekw=None, **inputs):
    in_maps, cpw = _host_prep(inputs)
    if cpw not in _PROG_CACHE:
        _PROG_CACHE[cpw] = _build_program(cpw)
    nc = _PROG_CACHE[cpw]
    kw = dict(_tracekw or {})
    res = run_bass_kernel_spmd(nc, in_maps, core_ids=list(range(NCORE)),
                               trace=_trace, **kw)
    out = res.results[0]["out"].astype(np.float32)
    if _trace:
        return out, res
    return out


# revision 27
# speedup vs baseline: 1.0120x; 1.0120x over previous
"""GATv2 4-layer GNN on 8 TRN2 NeuronCores.

Sharding: nodes LPT-packed into 240 balanced (core,window) bins of 128 nodes;
edges assigned to the bin of their dst. Per layer: local node matmuls ->
AllGather of bf16 xl -> per-window edge phase (indirect-DMA gather of xl[src],
one-hot matmuls for xr expansion and segment-softmax scatter into PSUM) ->
BN-stat AllReduce -> fused BN+elu. Final: graph mean-pool via indicator
matmuls + AllReduce + 2-layer MLP (replicated).
"""
import heapq

import numpy as np
import ml_dtypes

import concourse.bass as bass
import concourse.bacc as bacc
import concourse.mybir as mybir
import concourse.tile as tile
from concourse.bass_utils import run_bass_kernel_spmd
from concourse.masks import make_identity

P = 128
NCORE = 8
N = 30000
E = 300000
G = 128
IN = 128
HID = 64
HEADS = 4
NCLS = 10
EPS = 1e-5
SLOPE = 0.2
NPC = N // NCORE            # 3750 real nodes per core
W = 30                      # windows per core
NPAD = W * P                # 3840 padded nodes per core
ROWS = NCORE * NPAD         # 30720 rows in the allgathered xl
OC = HEADS * HID            # 256
# (layer, input-channel-chunk) order for weight / hT chunk layouts
IC_CHUNKS = [(0, 0), (1, 0), (1, 1), (2, 0), (2, 1), (3, 0), (3, 1)]
# (layer, output-channel-chunk) order for BN params/stats
OC_CHUNKS = [(0, 0), (0, 1), (1, 0), (1, 1), (2, 0), (2, 1), (3, 0)]

f32 = mybir.dt.float32
bf16 = mybir.dt.bfloat16
i32 = mybir.dt.int32
bf = ml_dtypes.bfloat16


def _pack_nodes(deg):
    """LPT-pack nodes into NCORE*W bins of <=128 nodes, balancing edge load."""
    nbins = NCORE * W
    order = np.argsort(-deg, kind="stable")
    heap = [(0, b) for b in range(nbins)]
    heapq.heapify(heap)
    counts = np.zeros(nbins, np.int64)
    load = np.zeros(nbins, np.int64)
    node_bin = np.empty(N, np.int64)
    node_pos = np.empty(N, np.int64)
    for n in order:
        while True:
            l, b = heapq.heappop(heap)
            if counts[b] < P:
                break
        node_bin[n] = b
        node_pos[n] = counts[b]
        counts[b] += 1
        load[b] += deg[n]
        if counts[b] < P:
            heapq.heappush(heap, (load[b], b))
    return node_bin, node_pos, int(load.max())


def _host_prep(inputs):
    x = np.asarray(inputs["x"], np.float32)
    ei = np.asarray(inputs["edge_index"], np.int64)
    batch = np.asarray(inputs["batch"], np.int64)

    src = ei[0]
    dst = ei[1]
    deg = np.bincount(dst, minlength=N) + 1  # +1 self-loop
    node_bin, node_pos, max_load = _pack_nodes(deg)
    nodes_per_bin = np.bincount(node_bin, minlength=NCORE * W)
    nonself_load = np.bincount(node_bin[dst], minlength=NCORE * W)
    cpw = 1 + max(1, -(-int(nonself_load.max()) // P))  # chunk0 = self loops
    nch = W * cpw                            # chunk columns per core
    row_of = (node_bin // W) * NPAD + (node_bin % W) * P + node_pos  # global row

    # per-edge (non-self): assigned bin = bin of dst
    e_bin = node_bin[dst]
    e_srcrow = row_of[src].astype(np.int32)
    e_dstpos = node_pos[dst].astype(np.int32)
    order = np.argsort(e_bin, kind="stable")
    sorted_bin = e_bin[order]
    counts_b = np.bincount(sorted_bin, minlength=NCORE * W)

    srcrow_tab = np.zeros((NCORE, P, nch), np.int32)
    dstf_tab = np.full((NCORE, P, nch), 999.0, bf)

    # chunk 0 of each window: self loop of node at position p (pad -> masked)
    cores_n = node_bin // W
    wins_n = node_bin % W
    srcrow_tab[cores_n, node_pos, wins_n * cpw] = row_of[np.arange(N)]
    dstf_tab[cores_n, node_pos, wins_n * cpw] = node_pos
    selfb_tab = np.full((NCORE, P, W), -1000.0, np.float32)
    selfb_tab[cores_n, node_pos, wins_n] = 0.0

    bin_starts = np.zeros(NCORE * W + 1, np.int64)
    np.cumsum(counts_b, out=bin_starts[1:])
    se_srcrow = e_srcrow[order]
    se_dstpos = e_dstpos[order]
    for b in range(NCORE * W):
        c, w = b // W, b % W
        lo, hi = bin_starts[b], bin_starts[b + 1]
        k = hi - lo
        sl = np.arange(k)
        ch = w * cpw + 1 + sl // P    # chunk column (after self chunk)
        pp = sl % P                   # partition
        srcrow_tab[c, pp, ch] = se_srcrow[lo:hi]
        dstf_tab[c, pp, ch] = se_dstpos[lo:hi]

    # graph indicator per core: [P pos, W*P] (win w block cols -> graph onehot)
    ind_tab = np.zeros((NCORE, P, W * P), bf)
    ind_tab[cores_n, node_pos, wins_n * P + batch] = 1.0

    # x transposed per core, permuted order: [IN, NPAD]
    xT = np.zeros((NCORE, IN, NPAD), bf)
    cols = wins_n * P + node_pos
    xT[cores_n, :, cols] = x.astype(bf)

    # weights (identical on all cores)
    wl_tab = np.zeros((P, len(IC_CHUNKS) * OC), bf)
    wr_tab = np.zeros((P, len(IC_CHUNKS) * OC), bf)
    for j, (l, cc) in enumerate(IC_CHUNKS):
        wl = np.asarray(inputs[f"Wl{l}"], np.float32)
        wr = np.asarray(inputs[f"Wr{l}"], np.float32)
        wl_tab[:, j * OC:(j + 1) * OC] = wl[cc * P:(cc + 1) * P, :].astype(bf)
        wr_tab[:, j * OC:(j + 1) * OC] = wr[cc * P:(cc + 1) * P, :].astype(bf)
    att_tab = np.zeros((P, 4 * 2 * OC), bf)
    for l in range(4):
        a = np.asarray(inputs[f"att{l}"], np.float32).reshape(1, OC)
        att_tab[:, l * 2 * OC:(l + 1) * 2 * OC] = np.tile(a, (P, 2))
    g_tab = np.ones((P, len(OC_CHUNKS)), np.float32)
    b_tab = np.zeros((P, len(OC_CHUNKS)), np.float32)
    for j, (l, cc) in enumerate(OC_CHUNKS):
        g = np.asarray(inputs[f"bng{l}"], np.float32)
        bb = np.asarray(inputs[f"bnb{l}"], np.float32)
        seg = g[cc * P:(cc + 1) * P]
        g_tab[: len(seg), j] = seg
        segb = bb[cc * P:(cc + 1) * P]
        b_tab[: len(segb), j] = segb

    fc1w = np.zeros((HID, HID), bf)
    fc1w[:, :] = np.asarray(inputs["fc1_w"], np.float32).astype(bf)
    fc2w = np.zeros((HID, NCLS), bf)
    fc2w[:, :] = np.asarray(inputs["fc2_w"], np.float32).astype(bf)
    fc1b_rep = np.tile(np.asarray(inputs["fc1_b"], np.float32).reshape(1, HID), (P, 1))
    fc2b_rep = np.tile(np.asarray(inputs["fc2_b"], np.float32).reshape(1, NCLS), (P, 1))
    gcnt = np.bincount(batch, minlength=G).astype(np.float32)
    cnt_recip = (1.0 / np.maximum(gcnt, 1.0)).reshape(P, 1).astype(np.float32)

    in_maps = []
    for c in range(NCORE):
        in_maps.append(dict(
            xT=np.ascontiguousarray(xT[c]),
            selfb_tab=np.ascontiguousarray(selfb_tab[c]),
            srcrow_tab=np.ascontiguousarray(srcrow_tab[c]),
            dstf_tab=np.ascontiguousarray(dstf_tab[c]),
            ind_tab=np.ascontiguousarray(ind_tab[c]),
            wl_tab=wl_tab, wr_tab=wr_tab, att_tab=att_tab,
            g_tab=g_tab, b_tab=b_tab,
            fc1w=fc1w, fc2w=fc2w, fc1b_rep=fc1b_rep, fc2b_rep=fc2b_rep,
            cnt_recip=cnt_recip,
        ))
    return in_maps, cpw


def _build_program(cpw):
    nch = W * cpw
    nc = bacc.Bacc(num_devices=NCORE, num_swdge_queues=4,
                   dynamic_dma_scratch_size=32768)
    ap = mybir.AluOpType
    act = mybir.ActivationFunctionType
    rg = [list(range(NCORE))]

    # --- external inputs ---
    xT_in = nc.dram_tensor("xT", [IN, NPAD], bf16, kind="ExternalInput")
    srcrow_in = nc.dram_tensor("srcrow_tab", [P, nch], i32, kind="ExternalInput")
    selfb_in = nc.dram_tensor("selfb_tab", [P, W], f32, kind="ExternalInput")
    dstf_in = nc.dram_tensor("dstf_tab", [P, nch], bf16, kind="ExternalInput")
    ind_in = nc.dram_tensor("ind_tab", [P, W * P], bf16, kind="ExternalInput")
    wl_in = nc.dram_tensor("wl_tab", [P, len(IC_CHUNKS) * OC], bf16, kind="ExternalInput")
    wr_in = nc.dram_tensor("wr_tab", [P, len(IC_CHUNKS) * OC], bf16, kind="ExternalInput")
    att_in = nc.dram_tensor("att_tab", [P, 8 * OC], bf16, kind="ExternalInput")
    g_in = nc.dram_tensor("g_tab", [P, len(OC_CHUNKS)], f32, kind="ExternalInput")
    b_in = nc.dram_tensor("b_tab", [P, len(OC_CHUNKS)], f32, kind="ExternalInput")
    fc1w_in = nc.dram_tensor("fc1w", [HID, HID], bf16, kind="ExternalInput")
    fc2w_in = nc.dram_tensor("fc2w", [HID, NCLS], bf16, kind="ExternalInput")
    fc1b_in = nc.dram_tensor("fc1b_rep", [P, HID], f32, kind="ExternalInput")
    fc2b_in = nc.dram_tensor("fc2b_rep", [P, NCLS], f32, kind="ExternalInput")
    cnt_in = nc.dram_tensor("cnt_recip", [P, 1], f32, kind="ExternalInput")
    out_dram = nc.dram_tensor("out", [G, NCLS], f32, kind="ExternalOutput")

    with tile.TileContext(nc) as tc:
        with (
            tc.tile_pool(name="persist", bufs=1) as pers,
            tc.tile_pool(name="sbuf", bufs=3) as sb,
            tc.tile_pool(name="psum", bufs=2, space="PSUM") as ps,
            tc.tile_pool(name="psacc", bufs=3, space="PSUM") as psacc,
            tc.tile_pool(name="dram", bufs=1, space="DRAM") as dr,
        ):
            # ---- persistent tiles / table loads ----
            ident_f = pers.tile([P, P], f32)
            make_identity(nc, ident_f[:])
            ident_bf = pers.tile([P, P], bf16)
            nc.vector.tensor_copy(ident_bf[:], ident_f[:])
            iota_row_i = pers.tile([P, P], i32)
            nc.gpsimd.iota(iota_row_i[:], pattern=[[1, P]], base=0, channel_multiplier=0)
            iota_row_f = pers.tile([P, P], bf16)
            nc.vector.tensor_copy(iota_row_f[:], iota_row_i[:])
            iota_col_i = pers.tile([P, 1], i32)
            nc.gpsimd.iota(iota_col_i[:], pattern=[[0, 1]], base=0, channel_multiplier=1)
            iota_col_f = pers.tile([P, 1], f32)
            nc.vector.tensor_copy(iota_col_f[:], iota_col_i[:])
            ones_row = pers.tile([1, P], bf16)
            nc.vector.memset(ones_row[:], 1.0)
            alpha_col = pers.tile([P, 1], f32)
            nc.vector.memset(alpha_col[:], SLOPE)
            ones_f = pers.tile([1, P], f32)
            nc.vector.memset(ones_f[:], 1.0)

            def load(name, handle, shape, dt):
                t = pers.tile(shape, dt, name=name)
                nc.sync.dma_start(t[:], handle[:, :])
                return t
            xT_sb = load("xT_sb", xT_in, [IN, NPAD], bf16)
            srcrow = load("srcrow", srcrow_in, [P, nch], i32)
            selfb = load("selfb", selfb_in, [P, W], f32)
            dstf = load("dstf", dstf_in, [P, nch], bf16)
            ind_sb = load("ind_sb", ind_in, [P, W * P], bf16)
            wl_sb = load("wl_sb", wl_in, [P, len(IC_CHUNKS) * OC], bf16)
            wr_sb = load("wr_sb", wr_in, [P, len(IC_CHUNKS) * OC], bf16)
            att_sb = load("att_sb", att_in, [P, 8 * OC], bf16)
            g_sb = load("g_sb", g_in, [P, len(OC_CHUNKS)], f32)
            b_sb = load("b_sb", b_in, [P, len(OC_CHUNKS)], f32)
            fc1w_sb = load("fc1w_sb", fc1w_in, [HID, HID], bf16)
            fc2w_sb = load("fc2w_sb", fc2w_in, [HID, NCLS], bf16)
            fc1b_sb = load("fc1b_sb", fc1b_in, [P, HID], f32)
            fc2b_sb = load("fc2b_sb", fc2b_in, [P, NCLS], f32)
            cnt_sb = load("cnt_sb", cnt_in, [P, 1], f32)

            hT_a = pers.tile([P, NPAD], bf16)      # h^T chunk c0:128
            hT_b = pers.tile([P, NPAD], bf16)      # h^T chunk c128:256
            hpre_a = pers.tile([P, NPAD], bf16)    # pre-BN h^T chunk 0
            hpre_b = pers.tile([P, NPAD], bf16)
            h3_pre = pers.tile([P, W * HID], f32)  # layer3 pre-BN, [n, c] layout
            h3_bn = pers.tile([P, W * HID], bf16)
            xr_all = pers.tile([P, W * OC], bf16)
            xl_all = pers.tile([P, W * OC], bf16)
            stat_s = pers.tile([P, 2 * W], f32)    # per-window sums, cc in {0,1}
            stat_q = pers.tile([P, 2 * W], f32)    # per-window sum-of-squares

            # DRAM scratch
            xl_loc = dr.tile([NPAD, OC], bf16)
            xl_fulls = [dr.tile([ROWS, OC], bf16, addr_space="Shared", name=f"xl_full{i}")
                        for i in range(4)]
            stats_i = dr.tile([P, 4], f32)
            stats_os = [dr.tile([P, 4], f32, addr_space="Shared", name=f"stats_o{i}")
                        for i in range(4)]
            pool_i = dr.tile([P, HID], f32)
            pool_o = dr.tile([P, HID], f32, addr_space="Shared")

            for l in range(4):
                ic_chunks = [j for j, (ll, _) in enumerate(IC_CHUNKS) if ll == l]
                oc_chunks = [j for j, (ll, _) in enumerate(OC_CHUNKS) if ll == l]
                hts = [xT_sb] if l == 0 else [hT_a, hT_b]

                # ---- node phase: xl = h @ Wl, xr = h @ Wr ----
                for w in range(W):
                    ps_xl = ps.tile([P, OC], f32, name="ps_xl", tag="psA")
                    ps_xr = ps.tile([P, OC], f32, name="ps_xr", tag="psB")
                    for i, j in enumerate(ic_chunks):
                        lhs = hts[i][:, w * P:(w + 1) * P]
                        nc.tensor.matmul(ps_xl[:], lhsT=lhs, rhs=wl_sb[:, j * OC:(j + 1) * OC],
                                         start=(i == 0), stop=(i == len(ic_chunks) - 1))
                        nc.tensor.matmul(ps_xr[:], lhsT=lhs, rhs=wr_sb[:, j * OC:(j + 1) * OC],
                                         start=(i == 0), stop=(i == len(ic_chunks) - 1))
                    xlsl = xl_all[:, w * OC:(w + 1) * OC]
                    nc.scalar.copy(xlsl, ps_xl[:])
                    nc.sync.dma_start(xl_loc[w * P:(w + 1) * P, :], xlsl)
                    nc.scalar.copy(xr_all[:, w * OC:(w + 1) * OC], ps_xr[:])

                # ---- allgather xl ----
                xl_full = xl_fulls[l]
                nc.gpsimd.collective_compute(
                    "AllGather", ap.bypass, replica_groups=rg,
                    ins=[xl_loc[:]], outs=[xl_full[:]],
                )

                # ---- edge phase ----
                attl = att_sb[:, l * 2 * OC:l * 2 * OC + OC]
                att2l = att_sb[:, l * 2 * OC:l * 2 * OC + 2 * OC]
                ngr = (cpw - 1) // 2
                for w in range(W):
                    acc = psacc.tile([P, OC + 4], f32, name="acc", tag="acc")
                    xrw = xr_all[:, w * OC:(w + 1) * OC]
                    # ---- chunk 0: self loops (identity one-hots) ----
                    kk0 = w * cpw
                    A0 = xl_all[:, w * OC:(w + 1) * OC]
                    z0 = sb.tile([P, OC], f32, name="z0", bufs=3)
                    nc.vector.tensor_tensor(out=z0[:], in0=A0, in1=xrw, op=ap.add)
                    z0l = sb.tile([P, OC], bf16, name="z0l", bufs=3)
                    nc.scalar.activation(z0l[:], z0[:], act.Prelu,
                                         bias=0.0, scale=1.0, alpha=alpha_col[:, :1])
                    zw0 = sb.tile([P, OC], bf16, name="zw0", bufs=3)
                    nc.vector.tensor_tensor(out=zw0[:], in0=z0l[:], in1=attl, op=ap.mult)
                    lg0 = sb.tile([P, 4], f32, name="lg0", bufs=3)
                    nc.vector.tensor_reduce(
                        out=lg0[:], in_=zw0[:].rearrange("p (h c) -> p h c", h=4),
                        axis=mybir.AxisListType.X, op=ap.add)
                    pa0 = sb.tile([P, OC + 4], bf16, name="pa0", bufs=3)
                    nc.scalar.activation(pa0[:, OC:OC + 4], lg0[:], act.Exp,
                                         bias=selfb[:, w:w + 1], scale=1.0)
                    nc.vector.tensor_tensor(
                        out=pa0[:, 0:OC].rearrange("p (h c) -> p h c", h=4),
                        in0=A0.rearrange("p (h c) -> p h c", h=4),
                        in1=pa0[:, OC:OC + 4][:, :, None].to_broadcast([P, 4, HID]),
                        op=ap.mult)
                    nc.tensor.matmul(acc[:], lhsT=ident_bf[:], rhs=pa0[:],
                                     start=True, stop=False)
                    # ---- groups of 2 regular chunks ----
                    for g in range(ngr):
                        kka = w * cpw + 1 + 2 * g
                        A2 = sb.tile([P, 2 * OC], bf16, name="A2", bufs=16)
                        psz = ps.tile([P, 2 * OC], f32, name="psz", tag="psA")
                        scs = []
                        for j in range(2):
                            kk = kka + j
                            g_inst = nc.gpsimd.indirect_dma_start(
                                out=A2[:, j * OC:(j + 1) * OC], out_offset=None,
                                in_=xl_full[:, :],
                                in_offset=bass.IndirectOffsetOnAxis(
                                    ap=srcrow[:, kk:kk + 1], axis=0),
                            )
                            qn = kk % 4
                            g_inst.ins.queue = f"qPoolDynamic{qn if qn else ''}"
                            sc_oh = sb.tile([P, P], bf16, name=f"sc_oh{j}", bufs=6)
                            nc.vector.tensor_tensor(
                                out=sc_oh[:], in0=dstf[:, kk:kk + 1].to_broadcast([P, P]),
                                in1=iota_row_f[:], op=ap.is_equal)
                            ps_dT = ps.tile([P, P], bf16, name="ps_dT", tag="psB")
                            nc.tensor.transpose(ps_dT[:], sc_oh[:], ident_bf[:])
                            ex_oh = sb.tile([P, P], bf16, name=f"ex_oh{j}", bufs=6)
                            nc.scalar.copy(ex_oh[:], ps_dT[:])
                            zsl = slice(j * OC, (j + 1) * OC)
                            nc.tensor.matmul(psz[:, zsl], lhsT=ex_oh[:], rhs=xrw,
                                             start=True, stop=False)
                            nc.tensor.matmul(psz[:, zsl], lhsT=ident_bf[:],
                                             rhs=A2[:, zsl], start=False, stop=True)
                            scs.append(sc_oh)
                        zl2 = sb.tile([P, 2 * OC], bf16, name="zl2", bufs=4)
                        nc.scalar.activation(zl2[:], psz[:], act.Prelu,
                                             bias=0.0, scale=1.0, alpha=alpha_col[:, :1])
                        zw2 = sb.tile([P, 2 * OC], bf16, name="zw2", bufs=3)
                        nc.vector.tensor_tensor(out=zw2[:], in0=zl2[:], in1=att2l, op=ap.mult)
                        lg2 = sb.tile([P, 8], f32, name="lg2", bufs=3)
                        nc.vector.tensor_reduce(
                            out=lg2[:], in_=zw2[:].rearrange("p (g h c) -> p g h c", g=2, h=4),
                            axis=mybir.AxisListType.X, op=ap.add)
                        pa2 = sb.tile([P, 2 * (OC + 4)], bf16, name="pa2", bufs=6)
                        pa2v = pa2[:].rearrange("p (g d) -> p g d", g=2)
                        nc.scalar.activation(
                            pa2v[:, :, OC:OC + 4], lg2[:].rearrange("p (g h) -> p g h", g=2),
                            act.Exp)
                        for j in range(2):
                            nc.vector.tensor_tensor(
                                out=pa2v[:, j, 0:OC].rearrange("p (h c) -> p h c", h=4),
                                in0=A2[:, j * OC:(j + 1) * OC]
                                    .rearrange("p (h c) -> p h c", h=4),
                                in1=pa2v[:, j, OC:OC + 4][:, :, None]
                                    .to_broadcast([P, 4, HID]),
                                op=ap.mult)
                            nc.tensor.matmul(
                                acc[:], lhsT=scs[j],
                                rhs=pa2v[:, j, :],
                                start=False,
                                stop=(g == ngr - 1 and j == 1))
                    # ---- window finalize ----
                    s_cl = sb.tile([P, 4], f32, name="s_cl")
                    nc.vector.tensor_scalar_max(s_cl[:], acc[:, OC:OC + 4], 1e-30)
                    r_t = sb.tile([P, 4], f32, name="r_t")
                    nc.vector.reciprocal(r_t[:], s_cl[:])
                    o_sb = sb.tile([P, OC], f32, name="o_sb")
                    nc.vector.tensor_tensor(
                        out=o_sb[:].rearrange("p (h c) -> p h c", h=4),
                        in0=acc[:, 0:OC].rearrange("p (h c) -> p h c", h=4),
                        in1=r_t[:, :, None].to_broadcast([P, 4, HID]),
                        op=ap.mult)
                    if l < 3:
                        for cc, hp in enumerate([hpre_a, hpre_b]):
                            ps_t = ps.tile([P, P], f32, name="ps_t", tag="psC", bufs=1)
                            nc.tensor.transpose(ps_t[:], o_sb[:, cc * P:(cc + 1) * P], ident_f[:])
                            nc.scalar.activation(
                                hp[:, w * P:(w + 1) * P], ps_t[:], act.Identity,
                                accum_out=stat_s[:, cc * W + w:cc * W + w + 1])
                            sq_t = sb.tile([P, P], f32, name="sq_t")
                            nc.scalar.activation(
                                sq_t[:], ps_t[:], act.Square,
                                accum_out=stat_q[:, cc * W + w:cc * W + w + 1])
                    else:
                        u1 = sb.tile([P, HID], f32, name="u1")
                        nc.vector.tensor_tensor(out=u1[:], in0=o_sb[:, 0:HID],
                                                in1=o_sb[:, HID:2 * HID], op=ap.add)
                        u2 = sb.tile([P, HID], f32, name="u2")
                        nc.vector.tensor_tensor(out=u2[:], in0=o_sb[:, 2 * HID:3 * HID],
                                                in1=o_sb[:, 3 * HID:4 * HID], op=ap.add)
                        u3 = sb.tile([P, HID], f32, name="u3")
                        nc.vector.tensor_tensor(out=u3[:], in0=u1[:], in1=u2[:], op=ap.add)
                        nc.vector.tensor_scalar_mul(
                            h3_pre[:, w * HID:(w + 1) * HID], u3[:], 0.25)
                        ps_t3 = ps.tile([HID, P], f32, name="ps_t3", tag="psC", bufs=1)
                        nc.tensor.transpose(ps_t3[:], h3_pre[:, w * HID:(w + 1) * HID], ident_f[:])
                        sq_t3 = sb.tile([HID, P], f32, name="sq_t3")
                        nc.scalar.activation(
                            sq_t3[:], ps_t3[:], act.Square,
                            accum_out=stat_q[:HID, w:w + 1])
                        nc.vector.tensor_reduce(
                            out=stat_s[:HID, w:w + 1], in_=ps_t3[:],
                            axis=mybir.AxisListType.X, op=ap.add)

                # ---- BN stats allreduce ----
                ncc = len(oc_chunks)
                st_sb = sb.tile([P, 4], f32, name="st_sb")
                if l < 3:
                    for cc in range(2):
                        nc.vector.tensor_reduce(
                            out=st_sb[:, cc:cc + 1], in_=stat_s[:, cc * W:(cc + 1) * W],
                            axis=mybir.AxisListType.X, op=ap.add)
                        nc.vector.tensor_reduce(
                            out=st_sb[:, 2 + cc:3 + cc], in_=stat_q[:, cc * W:(cc + 1) * W],
                            axis=mybir.AxisListType.X, op=ap.add)
                else:
                    nc.vector.tensor_reduce(
                        out=st_sb[:, 0:1], in_=stat_s[:, 0:W],
                        axis=mybir.AxisListType.X, op=ap.add)
                    nc.vector.tensor_reduce(
                        out=st_sb[:, 2:3], in_=stat_q[:, 0:W],
                        axis=mybir.AxisListType.X, op=ap.add)
                    nc.vector.memset(st_sb[:, 1:2], 0.0)
                    nc.vector.memset(st_sb[:, 3:4], 0.0)
                nc.sync.dma_start(stats_i[:], st_sb[:])
                nc.gpsimd.collective_compute(
                    "AllReduce", ap.add, replica_groups=rg,
                    ins=[stats_i[:]], outs=[stats_os[l][:]])
                gstats = sb.tile([P, 4], f32, name="gstats")
                nc.sync.dma_start(gstats[:], stats_os[l][:])

                # scale/shift: [P, 2] (cc cols)
                mu = sb.tile([P, 2], f32, name="mu")
                nc.vector.tensor_scalar_mul(mu[:], gstats[:, 0:2], 1.0 / N)
                msq = sb.tile([P, 2], f32, name="msq")
                nc.vector.tensor_scalar_mul(msq[:], gstats[:, 2:4], 1.0 / N)
                mu2 = sb.tile([P, 2], f32, name="mu2")
                nc.vector.tensor_tensor(out=mu2[:], in0=mu[:], in1=mu[:], op=ap.mult)
                var = sb.tile([P, 2], f32, name="var")
                nc.vector.tensor_tensor(out=var[:], in0=msq[:], in1=mu2[:], op=ap.subtract)
                vpe = sb.tile([P, 2], f32, name="vpe")
                nc.vector.tensor_scalar_add(vpe[:], var[:], EPS)
                rec = sb.tile([P, 2], f32, name="rec")
                nc.vector.reciprocal(rec[:], vpe[:])
                rstd = sb.tile([P, 2], f32, name="rstd")
                nc.scalar.sqrt(rstd[:], rec[:])
                scal = sb.tile([P, 2], f32, name="scal")
                shif = sb.tile([P, 2], f32, name="shif")
                for i, j in enumerate(oc_chunks):
                    nc.vector.tensor_tensor(out=scal[:, i:i + 1], in0=g_sb[:, j:j + 1],
                                            in1=rstd[:, i:i + 1], op=ap.mult)
                    tmp_ms = sb.tile([P, 1], f32, name="tmp_ms")
                    nc.vector.tensor_tensor(out=tmp_ms[:], in0=mu[:, i:i + 1],
                                            in1=scal[:, i:i + 1], op=ap.mult)
                    nc.vector.tensor_tensor(out=shif[:, i:i + 1], in0=b_sb[:, j:j + 1],
                                            in1=tmp_ms[:], op=ap.subtract)

                # ---- BN apply + elu ----
                if l < 3:
                    for cc, (hp, ht) in enumerate([(hpre_a, hT_a), (hpre_b, hT_b)]):
                        for hh in range(4):
                            hsl = slice(hh * (NPAD // 4), (hh + 1) * (NPAD // 4))
                            t_big = sb.tile([P, NPAD // 4], f32, name="t_big", bufs=2)
                            nc.vector.tensor_scalar(
                                out=t_big[:], in0=hp[:, hsl], scalar1=scal[:, cc:cc + 1],
                                scalar2=shif[:, cc:cc + 1], op0=ap.mult, op1=ap.add)
                            m_big = sb.tile([P, NPAD // 4], f32, name="m_big", bufs=2)
                            nc.vector.tensor_scalar_min(m_big[:], t_big[:], 0.0)
                            nc.scalar.activation(m_big[:], m_big[:], act.Exp)
                            nc.vector.tensor_scalar_add(m_big[:], m_big[:], -1.0)
                            nc.vector.tensor_tensor(out=ht[:, hsl], in0=t_big[:], in1=m_big[:], op=ap.max)
                else:
                    # replicate scale/shift rows: [P,1]->[1,P]->K=1 matmul
                    for nm, col in (("scal3", scal), ("shif3", shif)):
                        ps_r = ps.tile([1, P], f32, name="ps_r", tag="psC", bufs=1)
                        nc.tensor.transpose(ps_r[:], col[:, 0:1], ident_f[:])
                        row_t = sb.tile([1, P], f32, name=nm + "_row")
                        nc.scalar.copy(row_t[:], ps_r[:])
                        ps_rep = ps.tile([P, HID], f32, name="ps_rep", tag="psB")
                        nc.tensor.matmul(ps_rep[:], lhsT=ones_f[:, :P],
                                         rhs=row_t[:, 0:HID], start=True, stop=True)
                        rep_t = sb.tile([P, HID], f32, name=nm + "_rep", bufs=1)
                        nc.scalar.copy(rep_t[:], ps_rep[:])
                        if nm == "scal3":
                            scal3_rep = rep_t
                        else:
                            shif3_rep = rep_t
                    for w in range(W):
                        sl3 = slice(w * HID, (w + 1) * HID)
                        t3 = sb.tile([P, HID], f32, name="t3")
                        nc.vector.tensor_tensor(out=t3[:], in0=h3_pre[:, sl3],
                                                in1=scal3_rep[:], op=ap.mult)
                        nc.vector.tensor_tensor(out=t3[:], in0=t3[:],
                                                in1=shif3_rep[:], op=ap.add)
                        m3 = sb.tile([P, HID], f32, name="m3")
                        nc.vector.tensor_scalar_min(m3[:], t3[:], 0.0)
                        e3 = sb.tile([P, HID], f32, name="e3")
                        nc.scalar.activation(e3[:], m3[:], act.Exp)
                        nc.vector.tensor_scalar_add(e3[:], e3[:], -1.0)
                        nc.vector.tensor_tensor(out=h3_bn[:, sl3], in0=t3[:],
                                                in1=e3[:], op=ap.max)

            # ---- graph mean pool + MLP (replicated) ----
            ps_pool = psacc.tile([P, HID], f32, name="ps_pool", tag="acc")
            for w in range(W):
                nc.tensor.matmul(ps_pool[:], lhsT=ind_sb[:, w * P:(w + 1) * P],
                                 rhs=h3_bn[:, w * HID:(w + 1) * HID],
                                 start=(w == 0), stop=(w == W - 1))
            pool_sb = sb.tile([P, HID], f32, name="pool_sb")
            nc.vector.tensor_copy(pool_sb[:], ps_pool[:])
            nc.sync.dma_start(pool_i[:], pool_sb[:])
            nc.gpsimd.collective_compute(
                "AllReduce", ap.add, replica_groups=rg,
                ins=[pool_i[:]], outs=[pool_o[:]])
            pool_g = sb.tile([P, HID], f32, name="pool_g")
            nc.sync.dma_start(pool_g[:], pool_o[:])
            pooled = sb.tile([P, HID], f32, name="pooled")
            nc.vector.tensor_scalar_mul(pooled[:], pool_g[:], cnt_sb[:, :1])
            ps_pT = ps.tile([HID, P], f32, name="ps_pT", tag="psC", bufs=1)
            nc.tensor.transpose(ps_pT[:], pooled[:], ident_f[:])
            pooledT = sb.tile([HID, P], bf16, name="pooledT")
            nc.scalar.copy(pooledT[:], ps_pT[:])
            ps_o1 = ps.tile([P, HID], f32, name="ps_o1", tag="psA")
            nc.tensor.matmul(ps_o1[:], lhsT=pooledT[:], rhs=fc1w_sb[:, :], start=True, stop=True)
            o1b = sb.tile([P, HID], f32, name="o1b")
            nc.vector.tensor_tensor(out=o1b[:], in0=ps_o1[:], in1=fc1b_sb[:], op=ap.add)
            o1r = sb.tile([P, HID], bf16, name="o1r")
            nc.scalar.activation(o1r[:], o1b[:], act.Relu)
            o1rf = sb.tile([P, HID], f32, name="o1rf")
            nc.vector.tensor_copy(o1rf[:], o1r[:])
            ps_o1T = ps.tile([HID, P], f32, name="ps_o1T", tag="psB")
            nc.tensor.transpose(ps_o1T[:], o1rf[:], ident_f[:])
            o1T = sb.tile([HID, P], bf16, name="o1T")
            nc.scalar.copy(o1T[:], ps_o1T[:])
            ps_o2 = ps.tile([P, NCLS], f32, name="ps_o2", tag="psC", bufs=1)
            nc.tensor.matmul(ps_o2[:], lhsT=o1T[:], rhs=fc2w_sb[:, :], start=True, stop=True)
            o2b = sb.tile([P, NCLS], f32, name="o2b")
            nc.vector.tensor_tensor(out=o2b[:], in0=ps_o2[:], in1=fc2b_sb[:], op=ap.add)
            nc.sync.dma_start(out_dram[:, :], o2b[:])

    nc.compile()
    return nc


_PROG_CACHE = {}


def kernel(_trace=False, _tracekw=None, **inputs):
    in_maps, cpw = _host_prep(inputs)
    if cpw not in _PROG_CACHE:
        _PROG_CACHE[cpw] = _build_program(cpw)
    nc = _PROG_CACHE[cpw]
    kw = dict(_tracekw or {})
    res = run_bass_kernel_spmd(nc, in_maps, core_ids=list(range(NCORE)),
                               trace=_trace, **kw)
    out = res.results[0]["out"].astype(np.float32)
    if _trace:
        return out, res
    return out
